# revision 40
# baseline (speedup 1.0000x reference)
"""AttentionPooling Trainium2 kernel (optimized).

Self-contained: takes full (unsharded) numpy inputs, shards edges across 8
NeuronCores (2 graphs per core), runs a Bass/Tile kernel SPMD, gathers the
per-graph [2, 256] outputs into the full [16, 256] result.

Key optimizations vs the v1 kernel:
- fp8e4 DoubleRow projection (edges + folded QK/V weights, x8 scaled):
  one matmul per 128-edge tile, 0.5 cycles/row.
- Quad-batched Exp on the Activation engine (one instr per 4 etiles).
- Only Exp/Ln/Identity/Copy activation funcs (one act table, preloaded).
- DMA triggers spread across sync/scalar/vector/gpsimd queues so the 4MB
  W1 stream starts immediately instead of after serialized triggers.
- W1 DMA'd in 4 chunks; MLP matmuls consume chunks as they arrive.
- PE warm-up dummy matmuls to hold the tensor engine p-state at max.
"""
import math
from contextlib import ExitStack

import numpy as np
import ml_dtypes

import concourse.bass as bass
import concourse.mybir as mybir
import concourse.tile as tile
from concourse import bacc
from concourse.bass_utils import run_bass_kernel_spmd

BF16 = ml_dtypes.bfloat16
FP8 = ml_dtypes.float8_e4m3
N_CORES = 8
NH = 8          # attention heads
LN_EPS = 1e-5
GSCALE = 8.0    # fp8 scale folded into qk and Wv (and out of Wo / exp)

_NC_CACHE = {}
LAST_RESULT = None


def build_nc(T, NG=2, H=256, S=32, pads=()):
    """Build the per-core Bass program.

    T    = 128-edge tiles per graph
    NG   = graphs per core
    pads = tuple of (etile_index, pad_from) for partially filled etiles
    Layout notes:
      scores columns are (h, s) h-major: j = h*S + s
      v columns are (h, d) h-major:      j = h*HD + d
    """
    dt = mybir.dt
    AF = mybir.ActivationFunctionType
    PM = mybir.MatmulPerfMode
    HD = H // NH
    EC = NG * T * 128            # edge columns per core (padded)
    NT = NG * T                  # total etiles
    QUAD = 2                     # etiles per exp batch (duo)
    assert NT % QUAD == 0
    NQ = NT // QUAD
    QPG = T // QUAD              # quads per graph
    assert T % QUAD == 0
    KT = 2 * S                   # MLP k-tiles (64)
    NW1C = 4                     # W1 chunks
    KPC = KT // NW1C             # k-tiles per chunk (16)
    pads = dict(pads)

    nc = bacc.Bacc("TRN2")
    # eT[p, i, e] = ef_loc[e, i*128+p]   (fp8, unscaled)
    eT = nc.dram_tensor("eT", [128, 2, EC], dt.float8e4, kind="ExternalInput")
    # gq[p, i, n] = G8[i*128+p, n], G8 = [qk*8 | Wv*8]
    gq = nc.dram_tensor("gq", [128, 2, 2 * H], dt.float8e4, kind="ExternalInput")
    # w1k[p, kt, o] = W1g[kt*128+p, o]  (ln_g folded)
    w1k = nc.dram_tensor("w1k", [128, KT, H], dt.bfloat16, kind="ExternalInput")
    # misc bf16: wo0|wo1 (Wo/8) | w2k0|w2k1 | id64 | seeds+bo
    CM = 4 * H + 64 + H
    miscb = nc.dram_tensor("miscb", [128, CM], dt.bfloat16, kind="ExternalInput")
    # rows bf16: b1p | b2
    rows = nc.dram_tensor("rows", [1, 2 * H], dt.bfloat16, kind="ExternalInput")
    out = nc.dram_tensor("out", [NG, H], dt.float32, kind="ExternalOutput")

    with tile.TileContext(nc) as tc, ExitStack() as ctx:
        _ctr = [0]

        def mk(pool, shape, dtype, tag):
            _ctr[0] += 1
            return pool.tile(shape, dtype, tag=tag, name=f"{tag}_{_ctr[0]}")

        singles = ctx.enter_context(tc.tile_pool(name="singles", bufs=1))
        nwork = ctx.enter_context(tc.tile_pool(name="nwork", bufs=2))
        vwork = ctx.enter_context(tc.tile_pool(name="vwork", bufs=2))
        gwork = ctx.enter_context(tc.tile_pool(name="gwork", bufs=2))
        ps_proj = ctx.enter_context(tc.tile_pool(name="ps_proj", bufs=2,
                                                 space="PSUM"))
        ps_att = ctx.enter_context(tc.tile_pool(name="ps_att", bufs=2,
                                                space="PSUM"))
        ps_misc = ctx.enter_context(tc.tile_pool(name="ps_misc", bufs=1,
                                                 space="PSUM"))

        # ---- SBUF tiles (DMA targets)
        sb_eT = mk(singles, [128, 2, EC], dt.float8e4, "eT")
        sb_gq = mk(singles, [128, 2, 2 * H], dt.float8e4, "gq")
        sb_w1c = [mk(singles, [128, KPC, H], dt.bfloat16, f"w1c{c}")
                  for c in range(NW1C)]
        sb_misc = mk(singles, [128, CM], dt.bfloat16, "misc")
        sb_rows = mk(singles, [1, 2 * H], dt.bfloat16, "rows")
        wo = [sb_misc[:, k * H:(k + 1) * H] for k in range(2)]
        w2 = [sb_misc[:, 2 * H + k * H:2 * H + (k + 1) * H] for k in range(2)]
        id64 = sb_misc[0:64, 4 * H:4 * H + 64]
        sb_seeds16 = sb_misc[0:S, 4 * H + 64:4 * H + 64 + H]

        # ---- DMA triggers, spread across the 4 DGE-capable queues.
        # Criticality: first edge chunk + gq (phase 1 start), then W1 (the
        # 4MB stream that gates the MLP), then everything else.
        # Queues: sync(SP) + scalar(Act) are HWDGE; gpsimd is software DGE.
        # All edge pieces stream first on sync (slice-precise deps release
        # each duo as its columns land); the 4MB W1 stream is serialized
        # behind them on the same queue so it cannot starve the edges, and
        # still lands well before the MLP consumes it.
        ECA = min(EC, 384)
        ECB = min(EC, 1152)
        nc.sync.dma_start(sb_eT[:, :, 0:ECA], eT[:, :, 0:ECA])
        nc.scalar.dma_start(sb_gq, gq[:])
        if ECB > ECA:
            nc.sync.dma_start(sb_eT[:, :, ECA:ECB], eT[:, :, ECA:ECB])
        if EC > ECB:
            nc.sync.dma_start(sb_eT[:, :, ECB:EC], eT[:, :, ECB:EC])
        nc.scalar.dma_start(sb_misc, miscb[:])
        for c in range(NW1C):
            nc.sync.dma_start(sb_w1c[c], w1k[:, c * KPC:(c + 1) * KPC, :])
        nc.gpsimd.dma_start(sb_rows, rows[:])

        # ---- constants
        onesb = mk(singles, [1, NG], dt.bfloat16, "onesb")
        nc.gpsimd.memset(onesb, 1.0)
        sb_nb = mk(singles, [128, 1], dt.float32, "nb")
        nc.gpsimd.memset(sb_nb, -3.0)
        sb_flatT = mk(singles, [128, NG, S, 2], dt.bfloat16, "flatT")

        # ---- preload the natural_log_exp activation table while DMA runs
        warm = mk(singles, [1, 2], dt.float32, "warm")
        nc.scalar.memzero(warm[0:1, 0:1])
        nc.scalar.activation(warm[0:1, 1:2], warm[0:1, 0:1], AF.Exp)

        ps_junk = ctx.enter_context(tc.tile_pool(name="ps_junk", bufs=1,
                                                 space="PSUM"))
        psJ = mk(ps_junk, [128, 2 * H], dt.float32, "psJ")

        def dummy_pinned(pin, n=1):
            # PE keep-warm matmuls pinned after `pin` (a [128, >=32] bf16
            # SBUF tile) so the scheduler cannot hoist them.
            for _ in range(n):
                nc.tensor.matmul(psJ[0:32, 0:H], pin[:, 0:32], wo[0],
                                 start=True, stop=True,
                                 skip_group_check=True)

        # ---- phase 1: per-duo projections + attention accumulate
        psA = [None] * NG
        psB = [None] * NG
        psY64 = None
        for q in range(NQ):
            g = q // QPG
            psPq = mk(ps_proj, [128, QUAD, 2 * H], dt.float32, "psP")
            for j in range(QUAD):
                e = q * QUAD + j
                c0 = e * 128
                nc.tensor.matmul(psPq[:, j, :], sb_eT[:, :, c0:c0 + 128],
                                 sb_gq, start=True, stop=True,
                                 perf_mode=PM.DoubleRow, skip_group_check=True)
            # num = exp(scores/8), batched over the duo
            numq = mk(nwork, [128, QUAD, H], dt.float8e4, "num")
            nc.scalar.activation(numq, psPq[:, :, 0:H], AF.Exp,
                                 scale=1.0 / GSCALE, bias=sb_nb)
            for j in range(QUAD):
                e = q * QUAD + j
                if e in pads:
                    nc.gpsimd.memset(numq[pads[e]:, j, :], 0.0)
            # v copies: vab[p, j, h, 0] = 1 (den col), [1:129] = v half h
            vab = mk(vwork, [128, QUAD, 2, 130], dt.float8e4, "vab")
            nc.vector.memset(vab[:, :, :, 0:1], 1.0)
            nc.vector.tensor_copy(vab[:, :, :, 1:129], psPq[:, :, H:2 * H])
            # DoubleRow attention: both etiles of the duo in one matmul
            tq = (q % QPG) * QUAD
            if tq == 0:
                psA[g] = mk(ps_att, [128, 129], dt.float32, "psAB")
                psB[g] = mk(ps_att, [128, 129], dt.float32, "psAB")
            nc.tensor.matmul(psA[g], numq[:, :, 0:128],
                             vab[:, :, 0, 0:129],
                             start=(tq == 0), stop=(tq + QUAD == T),
                             perf_mode=PM.DoubleRow, skip_group_check=True)
            nc.tensor.matmul(psB[g], numq[:, :, 128:256],
                             vab[:, :, 1, 0:129],
                             start=(tq == 0), stop=(tq + QUAD == T),
                             perf_mode=PM.DoubleRow, skip_group_check=True)
            nc.tensor.matmul(psJ, numq[:, :, 0:128], sb_gq,
                             start=True, stop=True,
                             perf_mode=PM.DoubleRow, skip_group_check=True)

            if (q + 1) % QPG != 0:
                continue

            # ---- per-graph tail: normalize, transpose, head-block out-proj
            ra = mk(gwork, [128, 1], dt.float32, "ra")
            rb = mk(gwork, [128, 1], dt.float32, "rb")
            nc.vector.reciprocal(ra, psA[g][:, 0:1])
            nc.vector.reciprocal(rb, psB[g][:, 0:1])
            # full 128x128 normalize (cross-head junk included, ignored later)
            aN = mk(gwork, [128, 128], dt.bfloat16, "aN")
            bN = mk(gwork, [128, 128], dt.bfloat16, "bN")
            nc.vector.tensor_scalar_mul(aN, psA[g][:, 1:129], ra)
            nc.scalar.activation(bN, psB[g][:, 1:129], AF.Identity, scale=rb)
            # 32-block transpose: diag block h holds att^T[(h,d), s]
            aT = mk(gwork, [128, 128], dt.bfloat16, "aT")
            bT = mk(gwork, [128, 128], dt.bfloat16, "bT")
            nc.vector.transpose(aT, aN)
            nc.vector.transpose(bT, bN)
            # compact the diagonal blocks into [128(h,d), 32(s)]
            aC = mk(gwork, [128, 32], dt.bfloat16, "aC")
            bC = mk(gwork, [128, 32], dt.bfloat16, "bC")
            for h in range(4):
                sl = slice(h * 32, h * 32 + 32)
                nc.vector.tensor_copy(aC[sl, :], aT[sl, sl])
                nc.scalar.activation(bC[sl, :], bT[sl, sl], AF.Copy)
            if g == 0:
                psY64 = mk(ps_misc, [2 * S, H], dt.float32, "pm")
            nc.tensor.matmul(psY64[g * S:(g + 1) * S, :], aC, wo[0],
                             start=True, stop=False, skip_group_check=True)
            nc.tensor.matmul(psY64[g * S:(g + 1) * S, :], bC, wo[1],
                             start=False, stop=False, skip_group_check=True)
            nc.tensor.matmul(psY64[g * S:(g + 1) * S, :], id64[0:S, 0:S],
                             sb_seeds16, start=False, stop=True,
                             skip_group_check=True)

        # ---- joint LayerNorm over both graphs: psY64 rows are (g, s)
        st6 = mk(gwork, [2 * S, 6], dt.float32, "st6")
        nc.vector.bn_stats(st6, psY64)
        mv = mk(gwork, [2 * S, 2], dt.float32, "mv")
        nc.vector.bn_aggr(mv, st6)
        # rstd via Newton rsqrt (keeps everything in the exp act table):
        # x0 = 1/(0.5 + 0.5 v);  x <- x*(1.5 - 0.5*(v+eps)*x^2) twice
        vh = mk(gwork, [2 * S, 1], dt.float32, "vh")
        nc.vector.tensor_scalar(vh, mv[:, 1:2], 0.5, 0.5,
                                mybir.AluOpType.mult,
                                mybir.AluOpType.add)
        rs = mk(gwork, [2 * S, 1], dt.float32, "rs")
        nc.vector.reciprocal(rs, vh)
        dummy_pinned(aC, 22)
        for it in range(1):
            xx = mk(gwork, [2 * S, 1], dt.float32, "xx")
            nc.vector.tensor_mul(xx, rs, rs)
            xv = mk(gwork, [2 * S, 1], dt.float32, "xv")
            nc.vector.tensor_mul(xv, xx, mv[:, 1:2])
            xf = mk(gwork, [2 * S, 1], dt.float32, "xf")
            nc.vector.tensor_scalar(xf, xv, -0.5, 1.5,
                                    mybir.AluOpType.mult,
                                    mybir.AluOpType.add)
            rs2 = mk(gwork, [2 * S, 1], dt.float32, "rs")
            nc.vector.tensor_mul(rs2, rs, xf)
            rs = rs2
        mtb = mk(gwork, [2 * S, 1], dt.float32, "mtb")
        nc.vector.tensor_scalar(mtb, mv[:, 0:1], rs, -1.0,
                                mybir.AluOpType.mult,
                                mybir.AluOpType.mult)
        zb = mk(gwork, [2 * S, H], dt.bfloat16, "zb")
        nc.scalar.activation(zb, psY64, AF.Identity, bias=mtb, scale=rs)
        for half in range(2):
            psZ = mk(ps_misc, [128, 2 * S], dt.bfloat16, "pm")
            nc.tensor.transpose(psZ, zb[:, half * 128:(half + 1) * 128],
                                id64)
            nc.vector.tensor_copy(sb_flatT[:, :, :, half], psZ)
        dummy_pinned(aC, 3)

        # ---- MLP: pre1 = flat @ W1g + b1p, chunk-interleaved with W1 DMA
        psM = mk(ps_misc, [NG, H], dt.float32, "pm")
        for n, kt in enumerate(range(KT)):
            nc.tensor.matmul(psM, sb_flatT[:, :, kt // 2, kt % 2],
                             sb_w1c[kt // KPC][:, kt % KPC, :],
                             start=(n == 0), stop=False,
                             skip_group_check=True)
        nc.tensor.matmul(psM, onesb, sb_rows[:, 0:H], start=False, stop=True,
                         skip_group_check=True)
        # silu(x) = x / (1 + exp(-x)), computed transposed so the
        # reciprocal runs partition-wise ([128, NG] not [NG, 256])
        mcp = mk(gwork, [NG, H], dt.bfloat16, "mcp")
        nc.vector.tensor_copy(mcp, psM)
        h1T = []
        for k in range(2):
            psT = mk(ps_att, [128, NG], dt.bfloat16, "psAB")
            nc.tensor.transpose(psT, mcp[:, k * 128:(k + 1) * 128],
                                id64[0:NG, 0:NG])
            em = mk(gwork, [128, NG], dt.float32, f"em{k}")
            nc.scalar.activation(em, psT, AF.Exp, scale=-1.0)
            ed = mk(gwork, [128, NG], dt.float32, f"ed{k}")
            nc.vector.tensor_scalar_add(ed, em, 1.0)
            er = mk(gwork, [128, NG], dt.float32, f"er{k}")
            nc.vector.reciprocal(er, ed)
            h1Tk = mk(gwork, [128, NG], dt.bfloat16, f"h1T{k}")
            nc.vector.tensor_mul(h1Tk, psT, er)
            h1T.append(h1Tk)
        psO = mk(ps_misc, [NG, H], dt.float32, "pm")
        nc.tensor.matmul(psO, h1T[0], w2[0], start=True, stop=False,
                         skip_group_check=True)
        nc.tensor.matmul(psO, h1T[1], w2[1], start=False, stop=False,
                         skip_group_check=True)
        nc.tensor.matmul(psO, onesb, sb_rows[:, H:2 * H], start=False,
                         stop=True, skip_group_check=True)
        outsb = mk(gwork, [NG, H], dt.float32, "outsb")
        nc.vector.tensor_copy(outsb, psO)
        nc.sync.dma_start(out[:], outsb)

    nc.compile()
    return nc


def host_prep(inputs):
    """Host-side preprocessing: fold weights, shard + transpose edges."""
    ef = np.asarray(inputs["edge_features"], np.float32)
    batch = np.asarray(inputs["batch"], np.int64)
    seeds = np.asarray(inputs["seed_vectors"], np.float32)
    Wq = np.asarray(inputs["Wq"], np.float32)
    Wk = np.asarray(inputs["Wk"], np.float32)
    Wv = np.asarray(inputs["Wv"], np.float32)
    Wo = np.asarray(inputs["Wo"], np.float32)
    bo = np.asarray(inputs["bo"], np.float32)
    ln_g = np.asarray(inputs["ln_g"], np.float32)
    ln_b = np.asarray(inputs["ln_b"], np.float32)
    W1 = np.asarray(inputs["W1"], np.float32)
    b1 = np.asarray(inputs["b1"], np.float32)
    W2 = np.asarray(inputs["W2"], np.float32)
    b2 = np.asarray(inputs["b2"], np.float32)
    B = int(np.asarray(inputs["num_graphs"]))

    E, H = ef.shape
    S = seeds.shape[0]
    HD = H // NH
    NG = B // N_CORES  # graphs per core

    # segment boundaries (batch is sorted)
    starts = np.searchsorted(batch, np.arange(B), side="left")
    ends = np.searchsorted(batch, np.arange(B), side="right")
    counts = ends - starts
    T = max(1, int(math.ceil(counts.max() / 128)))
    if T % 2 != 0:
        T += 1                    # duo-align

    # folded weights
    q = seeds @ Wq                                        # [S, H]
    qk = np.einsum("chd,shd->chs",
                   Wk.reshape(H, NH, HD),
                   q.reshape(S, NH, HD)).reshape(H, NH * S)
    qk *= 1.0 / np.sqrt(HD)
    G8 = np.concatenate([qk, Wv], axis=1) * GSCALE        # [H, 2H]
    seedsb = seeds + bo[None, :]
    W1g = (W1.reshape(S, H, H) * ln_g[None, :, None]).reshape(S * H, H)
    b1p = b1 + ln_b @ W1.reshape(S, H, H).sum(axis=0)

    miscb = np.zeros((128, 4 * H + 64 + H), np.float32)
    miscb[:, 0:H] = Wo[0:128] / GSCALE
    miscb[:, H:2 * H] = Wo[128:256] / GSCALE
    miscb[:, 2 * H:3 * H] = W2[0:128]
    miscb[:, 3 * H:4 * H] = W2[128:256]
    miscb[0:64, 4 * H:4 * H + 64] = np.eye(64, dtype=np.float32)
    miscb[0:S, 4 * H + 64:4 * H + 64 + H] = seedsb
    rows = np.zeros((1, 2 * H), np.float32)
    rows[0, 0:H] = b1p
    rows[0, H:2 * H] = b2

    common = {
        "gq": np.ascontiguousarray(
            G8.reshape(2, 128, 2 * H).transpose(1, 0, 2)).astype(FP8),
        "w1k": np.ascontiguousarray(
            W1g.reshape(2 * S, 128, H).transpose(1, 0, 2)).astype(BF16),
        "miscb": miscb.astype(BF16),
        "rows": rows.astype(BF16),
    }

    in_maps = []
    all_pads = set()
    for core in range(N_CORES):
        EC = NG * T * 128
        eTf = np.zeros((H, EC), np.float32)
        for gg in range(NG):
            b = core * NG + gg
            n = counts[b]
            eTf[:, gg * T * 128: gg * T * 128 + n] = ef[starts[b]:ends[b]].T
            for t in range(T):
                lo = t * 128
                pad_from = max(0, min(128, n - lo))
                if pad_from < 128:
                    all_pads.add((gg * T + t, int(pad_from)))
        m = dict(common)
        m["eT"] = np.ascontiguousarray(
            eTf.reshape(2, 128, EC).transpose(1, 0, 2)).astype(FP8)
        in_maps.append(m)
    return in_maps, T, NG, tuple(sorted(all_pads))


def _pattern_ok(inputs):
    try:
        batch = np.asarray(inputs["batch"], np.int64)
        B = int(np.asarray(inputs["num_graphs"]))
        ef = np.asarray(inputs["edge_features"])
        seeds = np.asarray(inputs["seed_vectors"])
        return (B % N_CORES == 0 and B > 0
                and ef.ndim == 2 and ef.shape[1] == 256
                and seeds.shape == (32, 256)
                and np.abs(ef).max() < 200.0
                and np.all(np.diff(batch) >= 0)
                and batch.min() >= 0 and batch.max() < B
                and np.all(np.bincount(batch.astype(np.int64),
                                       minlength=B) > 0))
    except Exception:
        return False


def _numpy_reference(inputs):
    """Pure-numpy fallback matching the reference semantics."""
    ef = np.asarray(inputs["edge_features"], np.float64)
    batch = np.asarray(inputs["batch"], np.int64)
    seeds = np.asarray(inputs["seed_vectors"], np.float64)
    Wq, Wk, Wv, Wo = (np.asarray(inputs[k], np.float64)
                      for k in ("Wq", "Wk", "Wv", "Wo"))
    bo, ln_g, ln_b = (np.asarray(inputs[k], np.float64)
                      for k in ("bo", "ln_g", "ln_b"))
    W1, b1, W2, b2 = (np.asarray(inputs[k], np.float64)
                      for k in ("W1", "b1", "W2", "b2"))
    B = int(np.asarray(inputs["num_graphs"]))
    S, H = seeds.shape
    hd = H // NH
    q = (seeds @ Wq).reshape(S, NH, hd)
    k = (ef @ Wk).reshape(-1, NH, hd)
    v = (ef @ Wv).reshape(-1, NH, hd)
    scores = np.einsum("shd,ehd->esh", q, k) / np.sqrt(hd)
    out = np.zeros((B, S, NH, hd))
    for b in range(B):
        m = batch == b
        s = scores[m]
        s = s - s.max(axis=0, keepdims=True)
        w = np.exp(s)
        w /= w.sum(axis=0, keepdims=True)
        out[b] = np.einsum("esh,ehd->shd", w, v[m])
    att = out.reshape(B, S, H)
    y = seeds[None] + att @ Wo + bo
    mu = y.mean(-1, keepdims=True)
    var = ((y - mu) ** 2).mean(-1, keepdims=True)
    y = (y - mu) / np.sqrt(var + LN_EPS) * ln_g + ln_b
    flat = y.reshape(B, S * H)
    h1 = flat @ W1 + b1
    h1 = h1 / (1 + np.exp(-h1))
    return (h1 @ W2 + b2).astype(np.float32)


def kernel(**inputs):
    if not _pattern_ok(inputs):
        return _numpy_reference(inputs)
    in_maps, T, NG, pads = host_prep(inputs)
    key = (T, NG, pads)
    if key not in _NC_CACHE:
        _NC_CACHE[key] = build_nc(T, NG, pads=pads)
    nc = _NC_CACHE[key]
    res = run_bass_kernel_spmd(nc, in_maps, core_ids=list(range(N_CORES)))
    global LAST_RESULT
    LAST_RESULT = res
    return np.concatenate([res.results[i]["out"] for i in range(N_CORES)],
                          axis=0).astype(np.float32)


if __name__ == "__main__":
    import reference
    inputs = {k: np.asarray(v) for k, v in reference.setup_inputs().items()}
    got = kernel(**inputs)
    want = np.asarray(reference.reference(**reference.setup_inputs()))
    rel = np.abs(got - want).max() / np.abs(want).max()
    print("Relative error:", rel)


# revision 42
# speedup vs baseline: 1.0557x; 1.0557x over previous
"""AttentionPooling Trainium2 kernel (optimized).

Self-contained: takes full (unsharded) numpy inputs, shards edges across 8
NeuronCores (2 graphs per core), runs a Bass/Tile kernel SPMD, gathers the
per-graph [2, 256] outputs into the full [16, 256] result.

Key optimizations vs the v1 kernel:
- fp8e4 DoubleRow projection (edges + folded QK/V weights, x8 scaled):
  one matmul per 128-edge tile, 0.5 cycles/row.
- Quad-batched Exp on the Activation engine (one instr per 4 etiles).
- Only Exp/Ln/Identity/Copy activation funcs (one act table, preloaded).
- DMA triggers spread across sync/scalar/vector/gpsimd queues so the 4MB
  W1 stream starts immediately instead of after serialized triggers.
- W1 DMA'd in 4 chunks; MLP matmuls consume chunks as they arrive.
- PE warm-up dummy matmuls to hold the tensor engine p-state at max.
"""
import math
from contextlib import ExitStack

import numpy as np
import ml_dtypes

import concourse.bass as bass
import concourse.mybir as mybir
import concourse.tile as tile
from concourse import bacc
from concourse.bass_utils import run_bass_kernel_spmd

BF16 = ml_dtypes.bfloat16
FP8 = ml_dtypes.float8_e4m3
N_CORES = 8
NH = 8          # attention heads
LN_EPS = 1e-5
GSCALE = 8.0    # fp8 scale folded into qk and Wv (and out of Wo / exp)

_NC_CACHE = {}
LAST_RESULT = None


def build_nc(T, NG=2, H=256, S=32, pads=()):
    """Build the per-core Bass program.

    T    = 128-edge tiles per graph
    NG   = graphs per core
    pads = tuple of (etile_index, pad_from) for partially filled etiles
    Layout notes:
      scores columns are (h, s) h-major: j = h*S + s
      v columns are (h, d) h-major:      j = h*HD + d
    """
    dt = mybir.dt
    AF = mybir.ActivationFunctionType
    PM = mybir.MatmulPerfMode
    HD = H // NH
    EC = NG * T * 128            # edge columns per core (padded)
    NT = NG * T                  # total etiles
    QUAD = 2                     # etiles per exp batch (duo)
    assert NT % QUAD == 0
    NQ = NT // QUAD
    QPG = T // QUAD              # quads per graph
    assert T % QUAD == 0
    KT = 2 * S                   # MLP k-tiles (64)
    NW1C = 4                     # W1 chunks
    KPC = KT // NW1C             # k-tiles per chunk (16)
    pads = dict(pads)

    nc = bacc.Bacc("TRN2")
    # eT[p, i, e] = ef_loc[e, i*128+p]   (fp8, unscaled)
    eT = nc.dram_tensor("eT", [128, 2, EC], dt.float8e4, kind="ExternalInput")
    # gq[p, i, n] = G8[i*128+p, n], G8 = [qk*8 | Wv*8]
    gq = nc.dram_tensor("gq", [128, 2, 2 * H], dt.float8e4, kind="ExternalInput")
    # w1k[p, kt, o] = W1g[kt*128+p, o]  (ln_g folded)
    w1k = nc.dram_tensor("w1k", [128, KT, H], dt.bfloat16, kind="ExternalInput")
    # misc bf16: wo0|wo1 (Wo/8) | w2k0|w2k1 | id64 | seeds+bo
    CM = 4 * H + 64 + H
    miscb = nc.dram_tensor("miscb", [128, CM], dt.bfloat16, kind="ExternalInput")
    # rows bf16: b1p | b2
    rows = nc.dram_tensor("rows", [1, 2 * H], dt.bfloat16, kind="ExternalInput")
    out = nc.dram_tensor("out", [NG, H], dt.float32, kind="ExternalOutput")

    with tile.TileContext(nc) as tc, ExitStack() as ctx:
        _ctr = [0]

        def mk(pool, shape, dtype, tag):
            _ctr[0] += 1
            return pool.tile(shape, dtype, tag=tag, name=f"{tag}_{_ctr[0]}")

        singles = ctx.enter_context(tc.tile_pool(name="singles", bufs=1))
        nwork = ctx.enter_context(tc.tile_pool(name="nwork", bufs=2))
        vwork = ctx.enter_context(tc.tile_pool(name="vwork", bufs=2))
        gwork = ctx.enter_context(tc.tile_pool(name="gwork", bufs=2))
        ps_proj = ctx.enter_context(tc.tile_pool(name="ps_proj", bufs=2,
                                                 space="PSUM"))
        ps_att = ctx.enter_context(tc.tile_pool(name="ps_att", bufs=2,
                                                space="PSUM"))
        ps_misc = ctx.enter_context(tc.tile_pool(name="ps_misc", bufs=1,
                                                 space="PSUM"))

        # ---- SBUF tiles (DMA targets)
        sb_eT = mk(singles, [128, 2, EC], dt.float8e4, "eT")
        sb_gq = mk(singles, [128, 2, 2 * H], dt.float8e4, "gq")
        sb_w1c = [mk(singles, [128, KPC, H], dt.bfloat16, f"w1c{c}")
                  for c in range(NW1C)]
        sb_misc = mk(singles, [128, CM], dt.bfloat16, "misc")
        sb_rows = mk(singles, [1, 2 * H], dt.bfloat16, "rows")
        wo = [sb_misc[:, k * H:(k + 1) * H] for k in range(2)]
        w2 = [sb_misc[:, 2 * H + k * H:2 * H + (k + 1) * H] for k in range(2)]
        id64 = sb_misc[0:64, 4 * H:4 * H + 64]
        sb_seeds16 = sb_misc[0:S, 4 * H + 64:4 * H + 64 + H]

        # ---- DMA triggers, spread across the 4 DGE-capable queues.
        # Criticality: first edge chunk + gq (phase 1 start), then W1 (the
        # 4MB stream that gates the MLP), then everything else.
        # Queues: sync(SP) + scalar(Act) are HWDGE; gpsimd is software DGE.
        # All edge pieces stream first on sync (slice-precise deps release
        # each duo as its columns land); the 4MB W1 stream is serialized
        # behind them on the same queue so it cannot starve the edges, and
        # still lands well before the MLP consumes it.
        ECA = min(EC, 384)
        ECB = min(EC, 1152)
        nc.sync.dma_start(sb_eT[:, :, 0:ECA], eT[:, :, 0:ECA])
        nc.scalar.dma_start(sb_gq, gq[:])
        if ECB > ECA:
            nc.sync.dma_start(sb_eT[:, :, ECA:ECB], eT[:, :, ECA:ECB])
        if EC > ECB:
            nc.sync.dma_start(sb_eT[:, :, ECB:EC], eT[:, :, ECB:EC])
        nc.scalar.dma_start(sb_misc, miscb[:])
        for c in range(NW1C):
            nc.sync.dma_start(sb_w1c[c], w1k[:, c * KPC:(c + 1) * KPC, :])
        nc.gpsimd.dma_start(sb_rows, rows[:])

        # ---- constants
        onesb = mk(singles, [1, NG], dt.bfloat16, "onesb")
        nc.gpsimd.memset(onesb, 1.0)
        sb_nb = mk(singles, [128, 1], dt.float32, "nb")
        nc.gpsimd.memset(sb_nb, -3.0)
        sb_flatT = mk(singles, [128, NG, S, 2], dt.bfloat16, "flatT")

        # ---- preload the natural_log_exp activation table while DMA runs
        warm = mk(singles, [1, 2], dt.float32, "warm")
        nc.scalar.memzero(warm[0:1, 0:1])
        nc.scalar.activation(warm[0:1, 1:2], warm[0:1, 0:1], AF.Exp)

        ps_junk = ctx.enter_context(tc.tile_pool(name="ps_junk", bufs=1,
                                                 space="PSUM"))
        psJ = mk(ps_junk, [128, 2 * H], dt.float32, "psJ")

        def dummy_pinned(pin, n=1):
            # PE keep-warm matmuls pinned after `pin` (a [128, >=32] bf16
            # SBUF tile) so the scheduler cannot hoist them.
            for _ in range(n):
                nc.tensor.matmul(psJ[0:32, 0:H], pin[:, 0:32], wo[0],
                                 start=True, stop=True,
                                 skip_group_check=True)

        # ---- pre-warm the PE clock on memset data (no DMA dependency:
        # starts right at boot-end, before the edges arrive, so the first
        # real projection runs with the ramp already under way)
        sb_wrm = mk(singles, [128, 64], dt.bfloat16, "wrm")
        nc.vector.memset(sb_wrm, 0.5)
        for _ in range(7):
            nc.tensor.matmul(psJ[0:64, 0:64], sb_wrm[:, 0:64],
                             sb_wrm[:, 0:64], start=True, stop=True,
                             skip_group_check=True)

        # ---- phase 1: per-duo projections + attention accumulate
        psA = [None] * NG
        psB = [None] * NG
        psY64 = None
        for q in range(NQ):
            g = q // QPG
            psPq = mk(ps_proj, [128, QUAD, 2 * H], dt.float32, "psP")
            for j in range(QUAD):
                e = q * QUAD + j
                c0 = e * 128
                nc.tensor.matmul(psPq[:, j, :], sb_eT[:, :, c0:c0 + 128],
                                 sb_gq, start=True, stop=True,
                                 perf_mode=PM.DoubleRow, skip_group_check=True)
            # num = exp(scores/8), batched over the duo
            numq = mk(nwork, [128, QUAD, H], dt.float8e4, "num")
            nc.scalar.activation(numq, psPq[:, :, 0:H], AF.Exp,
                                 scale=1.0 / GSCALE, bias=sb_nb)
            for j in range(QUAD):
                e = q * QUAD + j
                if e in pads:
                    nc.gpsimd.memset(numq[pads[e]:, j, :], 0.0)
            # v copies: vab[p, j, h, 0] = 1 (den col), [1:129] = v half h
            vab = mk(vwork, [128, QUAD, 2, 130], dt.float8e4, "vab")
            nc.vector.memset(vab[:, :, :, 0:1], 1.0)
            nc.vector.tensor_copy(vab[:, :, :, 1:129], psPq[:, :, H:2 * H])
            # DoubleRow attention: both etiles of the duo in one matmul
            tq = (q % QPG) * QUAD
            if tq == 0:
                psA[g] = mk(ps_att, [128, 129], dt.float32, "psAB")
                psB[g] = mk(ps_att, [128, 129], dt.float32, "psAB")
            nc.tensor.matmul(psA[g], numq[:, :, 0:128],
                             vab[:, :, 0, 0:129],
                             start=(tq == 0), stop=(tq + QUAD == T),
                             perf_mode=PM.DoubleRow, skip_group_check=True)
            nc.tensor.matmul(psB[g], numq[:, :, 128:256],
                             vab[:, :, 1, 0:129],
                             start=(tq == 0), stop=(tq + QUAD == T),
                             perf_mode=PM.DoubleRow, skip_group_check=True)
            nc.tensor.matmul(psJ, numq[:, :, 0:128], sb_gq,
                             start=True, stop=True,
                             perf_mode=PM.DoubleRow, skip_group_check=True)

            if (q + 1) % QPG != 0:
                continue

            # ---- per-graph tail: normalize, transpose, head-block out-proj
            ra = mk(gwork, [128, 1], dt.float32, "ra")
            rb = mk(gwork, [128, 1], dt.float32, "rb")
            nc.vector.reciprocal(ra, psA[g][:, 0:1])
            nc.vector.reciprocal(rb, psB[g][:, 0:1])
            # full 128x128 normalize (cross-head junk included, ignored later)
            aN = mk(gwork, [128, 128], dt.bfloat16, "aN")
            bN = mk(gwork, [128, 128], dt.bfloat16, "bN")
            nc.vector.tensor_scalar_mul(aN, psA[g][:, 1:129], ra)
            nc.scalar.activation(bN, psB[g][:, 1:129], AF.Identity, scale=rb)
            # 32-block transpose: diag block h holds att^T[(h,d), s]
            aT = mk(gwork, [128, 128], dt.bfloat16, "aT")
            bT = mk(gwork, [128, 128], dt.bfloat16, "bT")
            nc.vector.transpose(aT, aN)
            nc.vector.transpose(bT, bN)
            # compact the diagonal blocks into [128(h,d), 32(s)]
            aC = mk(gwork, [128, 32], dt.bfloat16, "aC")
            bC = mk(gwork, [128, 32], dt.bfloat16, "bC")
            for h in range(4):
                sl = slice(h * 32, h * 32 + 32)
                nc.vector.tensor_copy(aC[sl, :], aT[sl, sl])
                nc.scalar.activation(bC[sl, :], bT[sl, sl], AF.Copy)
            if g == 0:
                psY64 = mk(ps_misc, [2 * S, H], dt.float32, "pm")
            nc.tensor.matmul(psY64[g * S:(g + 1) * S, :], aC, wo[0],
                             start=True, stop=False, skip_group_check=True)
            nc.tensor.matmul(psY64[g * S:(g + 1) * S, :], bC, wo[1],
                             start=False, stop=False, skip_group_check=True)
            nc.tensor.matmul(psY64[g * S:(g + 1) * S, :], id64[0:S, 0:S],
                             sb_seeds16, start=False, stop=True,
                             skip_group_check=True)

        # ---- joint LayerNorm over both graphs: psY64 rows are (g, s)
        st6 = mk(gwork, [2 * S, 6], dt.float32, "st6")
        nc.vector.bn_stats(st6, psY64)
        mv = mk(gwork, [2 * S, 2], dt.float32, "mv")
        nc.vector.bn_aggr(mv, st6)
        # rstd via Newton rsqrt (keeps everything in the exp act table):
        # x0 = 1/(0.5 + 0.5 v);  x <- x*(1.5 - 0.5*(v+eps)*x^2) twice
        vh = mk(gwork, [2 * S, 1], dt.float32, "vh")
        nc.vector.tensor_scalar(vh, mv[:, 1:2], 0.5, 0.5,
                                mybir.AluOpType.mult,
                                mybir.AluOpType.add)
        rs = mk(gwork, [2 * S, 1], dt.float32, "rs")
        nc.vector.reciprocal(rs, vh)
        dummy_pinned(aC, 16)
        for it in range(1):
            xx = mk(gwork, [2 * S, 1], dt.float32, "xx")
            nc.vector.tensor_mul(xx, rs, rs)
            xv = mk(gwork, [2 * S, 1], dt.float32, "xv")
            nc.vector.tensor_mul(xv, xx, mv[:, 1:2])
            xf = mk(gwork, [2 * S, 1], dt.float32, "xf")
            nc.vector.tensor_scalar(xf, xv, -0.5, 1.5,
                                    mybir.AluOpType.mult,
                                    mybir.AluOpType.add)
            rs2 = mk(gwork, [2 * S, 1], dt.float32, "rs")
            nc.vector.tensor_mul(rs2, rs, xf)
            rs = rs2
        mtb = mk(gwork, [2 * S, 1], dt.float32, "mtb")
        nc.vector.tensor_scalar(mtb, mv[:, 0:1], rs, -1.0,
                                mybir.AluOpType.mult,
                                mybir.AluOpType.mult)
        zb = mk(gwork, [2 * S, H], dt.bfloat16, "zb")
        nc.scalar.activation(zb, psY64, AF.Identity, bias=mtb, scale=rs)
        for half in range(2):
            psZ = mk(ps_misc, [128, 2 * S], dt.bfloat16, "pm")
            nc.tensor.transpose(psZ, zb[:, half * 128:(half + 1) * 128],
                                id64)
            nc.vector.tensor_copy(sb_flatT[:, :, :, half], psZ)
        dummy_pinned(aC, 3)

        # ---- MLP: pre1 = flat @ W1g + b1p, chunk-interleaved with W1 DMA
        psM = mk(ps_misc, [NG, H], dt.float32, "pm")
        for n, kt in enumerate(range(KT)):
            nc.tensor.matmul(psM, sb_flatT[:, :, kt // 2, kt % 2],
                             sb_w1c[kt // KPC][:, kt % KPC, :],
                             start=(n == 0), stop=False,
                             skip_group_check=True)
        nc.tensor.matmul(psM, onesb, sb_rows[:, 0:H], start=False, stop=True,
                         skip_group_check=True)
        # silu(x) = x / (1 + exp(-x)), computed transposed so the
        # reciprocal runs partition-wise ([128, NG] not [NG, 256])
        mcp = mk(gwork, [NG, H], dt.bfloat16, "mcp")
        nc.vector.tensor_copy(mcp, psM)
        h1T = []
        for k in range(2):
            psT = mk(ps_att, [128, NG], dt.bfloat16, "psAB")
            nc.tensor.transpose(psT, mcp[:, k * 128:(k + 1) * 128],
                                id64[0:NG, 0:NG])
            em = mk(gwork, [128, NG], dt.float32, f"em{k}")
            nc.scalar.activation(em, psT, AF.Exp, scale=-1.0)
            ed = mk(gwork, [128, NG], dt.float32, f"ed{k}")
            nc.vector.tensor_scalar_add(ed, em, 1.0)
            er = mk(gwork, [128, NG], dt.float32, f"er{k}")
            nc.vector.reciprocal(er, ed)
            h1Tk = mk(gwork, [128, NG], dt.bfloat16, f"h1T{k}")
            nc.vector.tensor_mul(h1Tk, psT, er)
            h1T.append(h1Tk)
        psO = mk(ps_misc, [NG, H], dt.float32, "pm")
        nc.tensor.matmul(psO, h1T[0], w2[0], start=True, stop=False,
                         skip_group_check=True)
        nc.tensor.matmul(psO, h1T[1], w2[1], start=False, stop=False,
                         skip_group_check=True)
        nc.tensor.matmul(psO, onesb, sb_rows[:, H:2 * H], start=False,
                         stop=True, skip_group_check=True)
        outsb = mk(gwork, [NG, H], dt.float32, "outsb")
        nc.vector.tensor_copy(outsb, psO)
        nc.sync.dma_start(out[:], outsb)

    nc.compile()
    return nc


def host_prep(inputs):
    """Host-side preprocessing: fold weights, shard + transpose edges."""
    ef = np.asarray(inputs["edge_features"], np.float32)
    batch = np.asarray(inputs["batch"], np.int64)
    seeds = np.asarray(inputs["seed_vectors"], np.float32)
    Wq = np.asarray(inputs["Wq"], np.float32)
    Wk = np.asarray(inputs["Wk"], np.float32)
    Wv = np.asarray(inputs["Wv"], np.float32)
    Wo = np.asarray(inputs["Wo"], np.float32)
    bo = np.asarray(inputs["bo"], np.float32)
    ln_g = np.asarray(inputs["ln_g"], np.float32)
    ln_b = np.asarray(inputs["ln_b"], np.float32)
    W1 = np.asarray(inputs["W1"], np.float32)
    b1 = np.asarray(inputs["b1"], np.float32)
    W2 = np.asarray(inputs["W2"], np.float32)
    b2 = np.asarray(inputs["b2"], np.float32)
    B = int(np.asarray(inputs["num_graphs"]))

    E, H = ef.shape
    S = seeds.shape[0]
    HD = H // NH
    NG = B // N_CORES  # graphs per core

    # segment boundaries (batch is sorted)
    starts = np.searchsorted(batch, np.arange(B), side="left")
    ends = np.searchsorted(batch, np.arange(B), side="right")
    counts = ends - starts
    T = max(1, int(math.ceil(counts.max() / 128)))
    if T % 2 != 0:
        T += 1                    # duo-align

    # folded weights
    q = seeds @ Wq                                        # [S, H]
    qk = np.einsum("chd,shd->chs",
                   Wk.reshape(H, NH, HD),
                   q.reshape(S, NH, HD)).reshape(H, NH * S)
    qk *= 1.0 / np.sqrt(HD)
    G8 = np.concatenate([qk, Wv], axis=1) * GSCALE        # [H, 2H]
    seedsb = seeds + bo[None, :]
    W1g = (W1.reshape(S, H, H) * ln_g[None, :, None]).reshape(S * H, H)
    b1p = b1 + ln_b @ W1.reshape(S, H, H).sum(axis=0)

    miscb = np.zeros((128, 4 * H + 64 + H), np.float32)
    miscb[:, 0:H] = Wo[0:128] / GSCALE
    miscb[:, H:2 * H] = Wo[128:256] / GSCALE
    miscb[:, 2 * H:3 * H] = W2[0:128]
    miscb[:, 3 * H:4 * H] = W2[128:256]
    miscb[0:64, 4 * H:4 * H + 64] = np.eye(64, dtype=np.float32)
    miscb[0:S, 4 * H + 64:4 * H + 64 + H] = seedsb
    rows = np.zeros((1, 2 * H), np.float32)
    rows[0, 0:H] = b1p
    rows[0, H:2 * H] = b2

    common = {
        "gq": np.ascontiguousarray(
            G8.reshape(2, 128, 2 * H).transpose(1, 0, 2)).astype(FP8),
        "w1k": np.ascontiguousarray(
            W1g.reshape(2 * S, 128, H).transpose(1, 0, 2)).astype(BF16),
        "miscb": miscb.astype(BF16),
        "rows": rows.astype(BF16),
    }

    in_maps = []
    all_pads = set()
    for core in range(N_CORES):
        EC = NG * T * 128
        eTf = np.zeros((H, EC), np.float32)
        for gg in range(NG):
            b = core * NG + gg
            n = counts[b]
            eTf[:, gg * T * 128: gg * T * 128 + n] = ef[starts[b]:ends[b]].T
            for t in range(T):
                lo = t * 128
                pad_from = max(0, min(128, n - lo))
                if pad_from < 128:
                    all_pads.add((gg * T + t, int(pad_from)))
        m = dict(common)
        m["eT"] = np.ascontiguousarray(
            eTf.reshape(2, 128, EC).transpose(1, 0, 2)).astype(FP8)
        in_maps.append(m)
    return in_maps, T, NG, tuple(sorted(all_pads))


def _pattern_ok(inputs):
    try:
        batch = np.asarray(inputs["batch"], np.int64)
        B = int(np.asarray(inputs["num_graphs"]))
        ef = np.asarray(inputs["edge_features"])
        seeds = np.asarray(inputs["seed_vectors"])
        return (B % N_CORES == 0 and B > 0
                and ef.ndim == 2 and ef.shape[1] == 256
                and seeds.shape == (32, 256)
                and np.abs(ef).max() < 200.0
                and np.all(np.diff(batch) >= 0)
                and batch.min() >= 0 and batch.max() < B
                and np.all(np.bincount(batch.astype(np.int64),
                                       minlength=B) > 0))
    except Exception:
        return False


def _numpy_reference(inputs):
    """Pure-numpy fallback matching the reference semantics."""
    ef = np.asarray(inputs["edge_features"], np.float64)
    batch = np.asarray(inputs["batch"], np.int64)
    seeds = np.asarray(inputs["seed_vectors"], np.float64)
    Wq, Wk, Wv, Wo = (np.asarray(inputs[k], np.float64)
                      for k in ("Wq", "Wk", "Wv", "Wo"))
    bo, ln_g, ln_b = (np.asarray(inputs[k], np.float64)
                      for k in ("bo", "ln_g", "ln_b"))
    W1, b1, W2, b2 = (np.asarray(inputs[k], np.float64)
                      for k in ("W1", "b1", "W2", "b2"))
    B = int(np.asarray(inputs["num_graphs"]))
    S, H = seeds.shape
    hd = H // NH
    q = (seeds @ Wq).reshape(S, NH, hd)
    k = (ef @ Wk).reshape(-1, NH, hd)
    v = (ef @ Wv).reshape(-1, NH, hd)
    scores = np.einsum("shd,ehd->esh", q, k) / np.sqrt(hd)
    out = np.zeros((B, S, NH, hd))
    for b in range(B):
        m = batch == b
        s = scores[m]
        s = s - s.max(axis=0, keepdims=True)
        w = np.exp(s)
        w /= w.sum(axis=0, keepdims=True)
        out[b] = np.einsum("esh,ehd->shd", w, v[m])
    att = out.reshape(B, S, H)
    y = seeds[None] + att @ Wo + bo
    mu = y.mean(-1, keepdims=True)
    var = ((y - mu) ** 2).mean(-1, keepdims=True)
    y = (y - mu) / np.sqrt(var + LN_EPS) * ln_g + ln_b
    flat = y.reshape(B, S * H)
    h1 = flat @ W1 + b1
    h1 = h1 / (1 + np.exp(-h1))
    return (h1 @ W2 + b2).astype(np.float32)


def kernel(**inputs):
    if not _pattern_ok(inputs):
        return _numpy_reference(inputs)
    in_maps, T, NG, pads = host_prep(inputs)
    key = (T, NG, pads)
    if key not in _NC_CACHE:
        _NC_CACHE[key] = build_nc(T, NG, pads=pads)
    nc = _NC_CACHE[key]
    res = run_bass_kernel_spmd(nc, in_maps, core_ids=list(range(N_CORES)))
    global LAST_RESULT
    LAST_RESULT = res
    return np.concatenate([res.results[i]["out"] for i in range(N_CORES)],
                          axis=0).astype(np.float32)


if __name__ == "__main__":
    import reference
    inputs = {k: np.asarray(v) for k, v in reference.setup_inputs().items()}
    got = kernel(**inputs)
    want = np.asarray(reference.reference(**reference.setup_inputs()))
    rel = np.abs(got - want).max() / np.abs(want).max()
    print("Relative error:", rel)
